# revision 9
# baseline (speedup 1.0000x reference)
"""CLIP contrastive loss (nn_ClipLoss) on 8 Trainium2 NeuronCores.

Strategy (row-sharded data parallel, fp8 DoubleRow matmul):
  - Each core k holds its row shard of the image embeddings plus the full
    normalized text matrix transposed (txtT), per the sharding hint: the
    "all-gathered normalized text embeddings" are materialized host-side
    (full_io staging), normalized in f32 and quantized to fp8 e4m3.  Text
    values are pre-scaled by 16 to clear the e4m3 subnormal range; the 1/16
    rides the per-row exp scale.
  - Logits block per core: [1024, 8192] = aT.T @ bT computed in fp8 with
    MatmulPerfMode.DoubleRowSwInterleave: each instruction contracts TWO
    K=128 slices at 0.5 cycles/row (4x bf16 MACs/cycle).  The hw-native
    weight layout (A/B pairs interleaved per column, columns reversed) is
    produced on the host for the stationary image tiles; the moving text
    tiles use the plain [p, 2, n] access pattern.
  - Image-side normalization stays on device: per-row-chunk squared norms
    via fused DVE scalar_tensor_tensor (free-dim accum), then
    ra = 1/(16*T*||a||) via ACT Ln/Exp.  ra rides the ACT Exp `scale`
    per-partition operand so the matmul never waits on a norm chain.
  - exp on ACT with fused row-sum (accum_out).  Column partial sums
    accumulate on DVE in bf16 (2x mode); per-block partition reduction via
    ones-matmul into a dedicated PSUM bank, DMA'd straight to the
    ReduceScatter staging buffer.
  - One 32KB ReduceScatter hands each core its own column shard; each core
    emits one fp32 partial; host sums 8 partials.
"""

import math

import numpy as np
import ml_dtypes

N_FULL = 8192
D_FULL = 1024
W = 8
P = 128
NS = 512
TEMP = 0.07
TXT_PRESCALE = 16.0
LN_SCALE = math.log(1.0 / (TXT_PRESCALE * TEMP))

_CACHE: dict = {}


def build_bass(n_global: int = N_FULL, d: int = D_FULL, collectives: bool = True):
    """Build the SPMD bass program (identical on all cores).

    collectives=False replaces the ReduceScatter with a local DMA stand-in
    (for single-core TimelineSim cost modeling only — numerically wrong
    across cores, but dependency/traffic equivalent on one core).
    """
    from contextlib import ExitStack

    import concourse.mybir as mybir
    import concourse.tile as tile
    from concourse import bacc

    f32 = mybir.dt.float32
    bf16 = mybir.dt.bfloat16
    fp8 = mybir.dt.float8e4
    AF = mybir.ActivationFunctionType
    OP = mybir.AluOpType
    X = mybir.AxisListType.X
    PM = mybir.MatmulPerfMode.DoubleRowSwInterleave

    m_loc = n_global // W          # rows per core
    mc_n = m_loc // P              # image row chunks per core
    dc_n = d // P                  # K chunks
    pc_n = dc_n // 2               # K pair-chunks (DoubleRow)
    gw = m_loc                     # column-block width (= shard width)
    ng = n_global // gw            # column blocks
    ns = min(NS, gw)               # psum slice width (one bank)
    n2_n = gw // ns

    import concourse.bacc as bacc_mod

    if not getattr(bacc_mod, "_clip_act_tables_patched", False):
        _orig_tabs = bacc_mod.get_activation_tables

        def _one_set_tables(module_arch):
            tabs = dict(_orig_tabs(module_arch))
            full_name = "natural_log_exp_and_others"
            if full_name in tabs:
                ours = {AF.Ln, AF.Exp, AF.Copy, AF.Identity, AF.Square}
                for name in tabs:
                    if name != full_name:
                        tabs[name] = set(tabs[name]) - ours
            return tabs

        bacc_mod.get_activation_tables = _one_set_tables
        bacc_mod._clip_act_tables_patched = True

    # All inputs are staged host-side in tile layout so every DMA moves
    # contiguous 8KB-per-partition lines (128 descriptors per tile):
    #   img  [P, mc_n*d]  : [p, m*d + k]    = image[m*128+p, k]       (bf16)
    #   txtn [P, mc_n*d]  : same layout for normalized*16 text        (bf16)
    #   imgT [P, dc_n*m]  : [p, c*m_loc+mi] = interleave(imgT)[c*128+p, mi]
    #   txtT [ng*P, dc_n*gw] : [g*128+p, c*gw+n] = txtT_n[c*128+p, g*gw+n]
    nc = bacc.Bacc("TRN2", target_bir_lowering=False, num_devices=W)
    img = nc.dram_tensor("img", [P, mc_n * d], bf16, kind="ExternalInput")
    txtn = nc.dram_tensor("txtn", [P, mc_n * d], bf16, kind="ExternalInput")
    imgT = nc.dram_tensor("imgT", [P, dc_n * m_loc], fp8, kind="ExternalInput")
    txtT = nc.dram_tensor("txtT", [ng * P, dc_n * gw], fp8, kind="ExternalInput")
    out_d = nc.dram_tensor("partial", [1, 1], f32, kind="ExternalOutput")
    rg = [list(range(W))]

    with tile.TileContext(nc) as tc, ExitStack() as ctx:
        sb = ctx.enter_context(tc.tile_pool(name="sb", bufs=1))
        ps = ctx.enter_context(tc.tile_pool(name="ps", bufs=1, space="PSUM"))
        dram = ctx.enter_context(tc.tile_pool(name="dram", bufs=1, space="DRAM"))

        # constants
        ones_bf = sb.tile([P, 1], bf16, name="ones_bf")
        nc.gpsimd.memset(ones_bf[:], 1.0)
        ones_f32 = sb.tile([P, 1], f32, name="ones_f32")
        nc.gpsimd.memset(ones_f32[:], 1.0)
        ln_invt = sb.tile([P, 1], f32, name="ln_invt")
        nc.gpsimd.memset(ln_invt[:], LN_SCALE)

        # collective DRAM buffers
        cc_rs_in = dram.tile([1, n_global], f32, name="cc_rs_in")
        cc_rs_out = dram.tile([1, m_loc], f32, name="cc_rs_out")

        # ---------------- prologue ------------------------------------------
        # Stationary image tiles (host-interleaved fp8) on the gpsimd queue;
        # the first moving text block rides the SP queue in parallel.
        aT = sb.tile([P, dc_n, m_loc], fp8, name="aT")
        nc.gpsimd.dma_start(
            aT[:], imgT[:].rearrange("p (c m) -> p c m", c=dc_n)
        )

        # a_nat chunks split so ra_0 is ready before the first exp
        a_nat = sb.tile([P, mc_n, d], bf16, name="a_nat")
        nat_splits = [(0, 1), (1, 2), (2, mc_n // 2), (mc_n // 2, mc_n)]
        for lo, hi in nat_splits:
            if hi > lo:
                nc.scalar.dma_start(
                    a_nat[:, lo:hi, :],
                    img[:, lo * d:hi * d].rearrange("p (m k) -> p m k", m=hi - lo),
                )
        b_nat = sb.tile([P, mc_n, d], bf16, name="b_nat")
        nc.gpsimd.dma_start(
            b_nat[:], txtn[:].rearrange("p (m k) -> p m k", m=mc_n)
        )

        # image norms on DVE (fused square + free-dim accum)
        norms2_a = sb.tile([P, mc_n], f32, name="norms2_a")
        for m in range(mc_n):
            sqa = sb.tile([P, d], bf16, name="sqa", tag="sqa", bufs=2)
            nc.vector.scalar_tensor_tensor(
                out=sqa[:], in0=a_nat[:, m, :], scalar=1.0, in1=a_nat[:, m, :],
                op0=OP.mult, op1=OP.mult, accum_out=norms2_a[:, m:m + 1],
            )
        ln_a = sb.tile([P, mc_n], f32, name="ln_a")
        ra_act = sb.tile([P, mc_n], f32, name="ra_act")

        d_nat = sb.tile([P, mc_n], f32, name="d_nat")
        row_acc = sb.tile([P, ng * mc_n], f32, name="row_acc")

        # ---------------- main loop over column blocks -----------------------
        for g in range(ng):
            bT = sb.tile([P, dc_n, gw], fp8, name="bT", tag="bT", bufs=3)
            nc.sync.dma_start(
                bT[:],
                txtT[g * P:(g + 1) * P, :].rearrange("p (c n) -> p c n", c=dc_n),
            )
            col_acc = sb.tile([P, gw], bf16, name="col_acc", tag="col", bufs=2)
            for m in range(mc_n):
                mm = ps.tile([P, gw], f32, name="mm", tag="mm", bufs=3)
                for n2 in range(n2_n):
                    for pc in range(pc_n):
                        nc.tensor.matmul(
                            mm[:, n2 * ns:(n2 + 1) * ns],
                            aT[:, 2 * pc:2 * pc + 2, m * P:(m + 1) * P],
                            bT[:, 2 * pc:2 * pc + 2, n2 * ns:(n2 + 1) * ns],
                            start=(pc == 0), stop=(pc == pc_n - 1),
                            perf_mode=PM,
                        )
                if g == 0:
                    # ra(m) interleaved into ACT program order just before
                    # its first use, so exp(g0, m0) isn't queued behind
                    # stats for later chunks.
                    nc.scalar.activation(
                        ln_a[:, m:m + 1], norms2_a[:, m:m + 1], AF.Ln
                    )
                    nc.scalar.activation(
                        ra_act[:, m:m + 1], ln_a[:, m:m + 1], AF.Exp,
                        scale=-0.5, bias=ln_invt[:],
                    )
                exp_t = sb.tile([P, gw], bf16, name="exp_t", tag="exp", bufs=4)
                slot = g * mc_n + m
                nc.scalar.activation(
                    exp_t[:], mm[:], AF.Exp,
                    scale=ra_act[:, m:m + 1],
                    accum_out=row_acc[:, slot:slot + 1],
                )
                if m == 0:
                    nc.vector.tensor_copy(col_acc[:], exp_t[:])
                else:
                    nc.vector.tensor_add(col_acc[:], col_acc[:], exp_t[:])

            # diag dot products spread across blocks (DVE slack mid-flight)
            if g >= 1:
                for mm_d in ([g - 1] if g < ng - 1 else [ng - 2, ng - 1]):
                    prod = sb.tile([P, d], bf16, name="prod", tag="prod",
                                   bufs=2)
                    nc.vector.scalar_tensor_tensor(
                        out=prod[:], in0=a_nat[:, mm_d, :], scalar=1.0,
                        in1=b_nat[:, mm_d, :],
                        op0=OP.mult, op1=OP.mult,
                        accum_out=d_nat[:, mm_d:mm_d + 1],
                    )

            # column partition-reduce, staged through SBUF on DVE for the
            # ReduceScatter input
            cs_row = sb.tile([1, gw], f32, name="cs_row", tag="cs_row", bufs=2)
            for n2 in range(n2_n):
                cs = ps.tile([1, ns], f32, name="cs", tag="cs", bufs=1)
                nc.tensor.matmul(
                    cs[:], ones_bf[:], col_acc[:, n2 * ns:(n2 + 1) * ns],
                    start=True, stop=True,
                )
                nc.vector.tensor_copy(cs_row[0:1, n2 * ns:(n2 + 1) * ns], cs[:])
            nc.gpsimd.dma_start(
                cc_rs_in[0:1, g * gw:(g + 1) * gw], cs_row[:]
            )

        dterm = sb.tile([P, mc_n], f32, name="dterm")
        nc.vector.tensor_mul(dterm[:], d_nat[:], ra_act[:])

        # ---------------- epilogue -------------------------------------------
        if collectives:
            nc.gpsimd.collective_compute(
                "ReduceScatter",
                OP.add,
                replica_groups=rg,
                ins=[cc_rs_in[:].opt()],
                outs=[cc_rs_out[:].opt()],
            )
        else:
            nc.gpsimd.dma_start(cc_rs_out[:], cc_rs_in[0:1, 0:m_loc])

        # my column shard's summed exp: [P, mc_n] (element order irrelevant)
        scol = sb.tile([P, mc_n], f32, name="scol")
        nc.gpsimd.dma_start(
            scol[:], cc_rs_out[0:1, :].rearrange("o (p f) -> p (o f)", p=P)
        )
        lsc = sb.tile([P, mc_n], f32, name="lsc")
        nc.scalar.activation(lsc[:], scol[:], AF.Ln)

        # total row sums: sum slots over g for each m
        srow = sb.tile([P, mc_n], f32, name="srow")
        nc.vector.tensor_reduce(
            srow[:],
            row_acc[:].rearrange("p (g m) -> p m g", g=ng),
            axis=X,
            op=OP.add,
        )
        lsr = sb.tile([P, mc_n], f32, name="lsr")
        nc.scalar.activation(lsr[:], srow[:], AF.Ln)

        # per-partition combine: F = 0.5*(sum lsr + sum lsc) - sum dterm
        s1 = sb.tile([P, 1], f32, name="s1")
        nc.vector.tensor_reduce(s1[:], lsr[:], axis=X, op=OP.add)
        s2 = sb.tile([P, 1], f32, name="s2")
        nc.vector.tensor_reduce(s2[:], lsc[:], axis=X, op=OP.add)
        s3 = sb.tile([P, 1], f32, name="s3")
        nc.vector.tensor_reduce(s3[:], dterm[:], axis=X, op=OP.add)
        tsum = sb.tile([P, 1], f32, name="tsum")
        nc.vector.tensor_add(tsum[:], s1[:], s2[:])
        fvec = sb.tile([P, 1], f32, name="fvec")
        nc.vector.scalar_tensor_tensor(
            out=fvec[:], in0=tsum[:], scalar=0.5, in1=s3[:],
            op0=OP.mult, op1=OP.subtract,
        )

        # partition sum -> scalar partial (scaled by 1/N)
        loss_ps = ps.tile([1, 1], f32, name="loss_ps", tag="cs", bufs=1)
        nc.tensor.matmul(loss_ps[:], ones_f32[:], fvec[:], start=True, stop=True)
        out_sb = sb.tile([1, 1], f32, name="out_sb")
        nc.scalar.mul(out_sb[:], loss_ps[:], 1.0 / n_global)
        nc.gpsimd.dma_start(out_d[0:1, 0:1], out_sb[:])

    nc.compile()
    return nc


def _interleave_weights(wT: np.ndarray) -> np.ndarray:
    """Host-side weight interleave for DoubleRowSwInterleave.

    wT: logical transposed weights [K, M] (fp8).  For each instruction
    slice (k-pair block pc, 128-col chunk mb), the hw expects byte
    fw = ci*128 + mi (ci = k-chunk within pair, mi = col within chunk) to
    hold W_s[p, 127 - jr] where s = fw % 2, jr = fw // 2.
    """
    K, M = wT.shape
    out = np.empty_like(wT)
    fw = np.arange(256)
    ci, mi = fw // 128, fw % 128
    s, jr = fw % 2, fw // 2
    j = 127 - jr
    for pc in range(K // 256):
        blk = wT[pc * 256:(pc + 1) * 256].reshape(2, 128, M // 128, 128)
        # dest [ci, p, mb, mi] = blk[s(fw), p, mb, j(fw)] with fw=ci*128+mi
        dest = blk[s, :, :, j]            # [256(fw), p, mb]
        dest = dest.transpose(1, 2, 0)    # [p, mb, fw]
        dest = dest.reshape(128, M // 128, 2, 128)  # [p, mb, ci, mi]
        out[pc * 256:(pc + 1) * 256] = (
            dest.transpose(2, 0, 1, 3).reshape(256, M)
        )
    return out


def _nat_tiled(x: np.ndarray) -> np.ndarray:
    """[m_loc, d] natural array -> [P, mc_n*d] tile layout [p, m*d+k]."""
    m_loc, d = x.shape
    return np.ascontiguousarray(
        x.reshape(m_loc // P, P, d).transpose(1, 0, 2).reshape(P, -1)
    )


def _kT_tiled(xT: np.ndarray) -> np.ndarray:
    """[K, cols] transposed array -> [P, (K//P)*cols] layout [p, c*cols+n]."""
    K, cols = xT.shape
    return np.ascontiguousarray(
        xT.reshape(K // P, P, cols).transpose(1, 0, 2).reshape(P, -1)
    )


def make_in_maps(image_embeddings: np.ndarray, text_embeddings: np.ndarray):
    n_global, d = image_embeddings.shape
    m_loc = n_global // W
    fp8 = ml_dtypes.float8_e4m3

    tn = text_embeddings / np.maximum(
        np.linalg.norm(text_embeddings, axis=1, keepdims=True), 1e-12
    )
    tn16 = (tn * TXT_PRESCALE).astype(np.float32)
    txtT_q = np.ascontiguousarray(tn16.T).astype(fp8)
    # txtT block-tiled: rows g*128+p, cols c*gw+n  (gw = m_loc)
    txtT_t = np.concatenate(
        [_kT_tiled(txtT_q[:, g * m_loc:(g + 1) * m_loc]) for g in range(W)],
        axis=0,
    )

    maps = []
    for k in range(W):
        sl = slice(k * m_loc, (k + 1) * m_loc)
        aT_q = np.ascontiguousarray(
            image_embeddings[sl].T.astype(np.float32)
        ).astype(fp8)
        maps.append({
            "img": _nat_tiled(image_embeddings[sl].astype(ml_dtypes.bfloat16)),
            "txtn": _nat_tiled(tn16[sl].astype(ml_dtypes.bfloat16)),
            "imgT": _kT_tiled(_interleave_weights(aT_q)),
            "txtT": txtT_t,
        })
    return maps


def kernel(image_embeddings: np.ndarray, text_embeddings: np.ndarray) -> np.ndarray:
    from concourse.bass_utils import run_bass_kernel_spmd

    n_global, d = image_embeddings.shape
    key = (n_global, d)
    if key not in _CACHE:
        _CACHE[key] = build_bass(n_global, d)
    nc = _CACHE[key]

    in_maps = make_in_maps(
        np.asarray(image_embeddings, np.float32),
        np.asarray(text_embeddings, np.float32),
    )
    res = run_bass_kernel_spmd(nc, in_maps, core_ids=list(range(W)))
    total = sum(float(r["partial"][0, 0]) for r in res.results)
    return np.asarray(total, dtype=np.float32)


# revision 13
# speedup vs baseline: 1.1016x; 1.1016x over previous
"""CLIP contrastive loss (nn_ClipLoss) on 8 Trainium2 NeuronCores.

Strategy (row-sharded data parallel, fp8 DoubleRow matmul):
  - Each core k holds its row shard of the image embeddings plus the full
    normalized text matrix transposed (txtT), per the sharding hint: the
    "all-gathered normalized text embeddings" are materialized host-side
    (full_io staging), normalized in f32 and quantized to fp8 e4m3.  Text
    values are pre-scaled by 16 to clear the e4m3 subnormal range; the 1/16
    rides the per-row exp scale.
  - Logits block per core: [1024, 8192] = aT.T @ bT computed in fp8 with
    MatmulPerfMode.DoubleRowSwInterleave: each instruction contracts TWO
    K=128 slices at 0.5 cycles/row (4x bf16 MACs/cycle).  The hw-native
    weight layout (A/B pairs interleaved per column, columns reversed) is
    produced on the host for the stationary image tiles; the moving text
    tiles use the plain [p, 2, n] access pattern.
  - Image-side normalization stays on device: per-row-chunk squared norms
    via fused DVE scalar_tensor_tensor (free-dim accum), then
    ra = 1/(16*T*||a||) via ACT Ln/Exp.  ra rides the ACT Exp `scale`
    per-partition operand so the matmul never waits on a norm chain.
  - exp on ACT with fused row-sum (accum_out).  Column partial sums
    accumulate on DVE in bf16 (2x mode); per-block partition reduction via
    ones-matmul into a dedicated PSUM bank, DMA'd straight to the
    ReduceScatter staging buffer.
  - One 32KB ReduceScatter hands each core its own column shard; each core
    emits one fp32 partial; host sums 8 partials.
"""

import math

import numpy as np
import ml_dtypes

N_FULL = 8192
D_FULL = 1024
W = 8
P = 128
NS = 512
TEMP = 0.07
TXT_PRESCALE = 16.0
LN_SCALE = math.log(1.0 / (TXT_PRESCALE * TEMP))

_CACHE: dict = {}


def _diag_sched(g: int, ng: int, mc_n: int) -> list:
    """Row chunks whose diagonal dot product runs at the end of block g:
    spread chunks over blocks [3, ng-2], two per block for the defaults."""
    first, last = 3, ng - 2
    nblk = max(last - first + 1, 1)
    per = -(-mc_n // nblk)
    if g < first or g > last:
        return []
    lo = (g - first) * per
    return list(range(lo, min(lo + per, mc_n)))


def build_bass(n_global: int = N_FULL, d: int = D_FULL, collectives: bool = True):
    """Build the SPMD bass program (identical on all cores).

    collectives=False replaces the ReduceScatter with a local DMA stand-in
    (for single-core TimelineSim cost modeling only — numerically wrong
    across cores, but dependency/traffic equivalent on one core).
    """
    from contextlib import ExitStack

    import concourse.mybir as mybir
    import concourse.tile as tile
    from concourse import bacc

    f32 = mybir.dt.float32
    bf16 = mybir.dt.bfloat16
    fp8 = mybir.dt.float8e4
    AF = mybir.ActivationFunctionType
    OP = mybir.AluOpType
    X = mybir.AxisListType.X
    PM = mybir.MatmulPerfMode.DoubleRowSwInterleave

    m_loc = n_global // W          # rows per core
    mc_n = m_loc // P              # image row chunks per core
    dc_n = d // P                  # K chunks
    pc_n = dc_n // 2               # K pair-chunks (DoubleRow)
    gw = m_loc                     # column-block width (= shard width)
    ng = n_global // gw            # column blocks
    ns = min(NS, gw)               # psum slice width (one bank)
    n2_n = gw // ns

    import concourse.bacc as bacc_mod

    if not getattr(bacc_mod, "_clip_act_tables_patched", False):
        _orig_tabs = bacc_mod.get_activation_tables

        def _one_set_tables(module_arch):
            tabs = dict(_orig_tabs(module_arch))
            full_name = "natural_log_exp_and_others"
            if full_name in tabs:
                ours = {AF.Ln, AF.Exp, AF.Copy, AF.Identity, AF.Square}
                for name in tabs:
                    if name != full_name:
                        tabs[name] = set(tabs[name]) - ours
            return tabs

        bacc_mod.get_activation_tables = _one_set_tables
        bacc_mod._clip_act_tables_patched = True

    # All inputs are staged host-side in tile layout so every DMA moves
    # contiguous 8KB-per-partition lines (128 descriptors per tile):
    #   img  [P, mc_n*d]  : [p, m*d + k]    = image[m*128+p, k]       (bf16)
    #   txtn [P, mc_n*d]  : same layout for normalized*16 text        (bf16)
    #   imgT [P, dc_n*m]  : [p, c*m_loc+mi] = interleave(imgT)[c*128+p, mi]
    #   txtT [ng*P, dc_n*gw] : [g*128+p, c*gw+n] = txtT_n[c*128+p, g*gw+n]
    nc = bacc.Bacc("TRN2", target_bir_lowering=False, num_devices=W)
    img = nc.dram_tensor("img", [P, mc_n * d], bf16, kind="ExternalInput")
    txtn = nc.dram_tensor("txtn", [P, mc_n * d], bf16, kind="ExternalInput")
    imgT = nc.dram_tensor("imgT", [P, dc_n * m_loc], fp8, kind="ExternalInput")
    txtT = nc.dram_tensor("txtT", [ng * P, dc_n * gw], fp8, kind="ExternalInput")
    out_d = nc.dram_tensor("partial", [1, 1], f32, kind="ExternalOutput")
    rg = [list(range(W))]

    with tile.TileContext(nc) as tc, ExitStack() as ctx:
        sb = ctx.enter_context(tc.tile_pool(name="sb", bufs=1))
        ps = ctx.enter_context(tc.tile_pool(name="ps", bufs=1, space="PSUM"))
        dram = ctx.enter_context(tc.tile_pool(name="dram", bufs=1, space="DRAM"))

        # constants
        ones_bf = sb.tile([P, 1], bf16, name="ones_bf")
        nc.gpsimd.memset(ones_bf[:], 1.0)
        ones_f32 = sb.tile([P, 1], f32, name="ones_f32")
        nc.gpsimd.memset(ones_f32[:], 1.0)
        ln_invt = sb.tile([P, 1], f32, name="ln_invt")
        nc.gpsimd.memset(ln_invt[:], LN_SCALE)

        # collective DRAM buffers
        cc_rs_in = dram.tile([1, n_global], f32, name="cc_rs_in")
        cc_rs_out = dram.tile([1, m_loc], f32, name="cc_rs_out")

        # ---------------- prologue ------------------------------------------
        # Stationary image tiles (host-interleaved fp8) on the gpsimd queue;
        # the first moving text block rides the SP queue in parallel.
        # The sim's DMA bus is a single serialized 341GB/s device, so arrival
        # order == dispatch order matters more than queue choice.  Everything
        # ordering-critical rides the SP queue in exactly the order of need:
        #   bT0, a_nat (3 chunks), bT1, bT2, b_nat (2 halves), bT3..bT7.
        # aT goes on gpsimd so its dispatch precedes bT0 on the bus.
        # ACT gets NO DMA dispatches: a DMACopy holds its SEQ until the
        # transfer completes, which would gate every exp behind it.
        aT = sb.tile([P, dc_n, m_loc], fp8, name="aT")
        nc.gpsimd.dma_start(
            aT[:], imgT[:].rearrange("p (c m) -> p c m", c=dc_n)
        )

        a_nat = sb.tile([P, mc_n, d], bf16, name="a_nat")
        b_nat = sb.tile([P, mc_n, d], bf16, name="b_nat")

        bT_tiles = []
        bT = sb.tile([P, dc_n, gw], fp8, name="bT", tag="bT", bufs=3)
        nc.sync.dma_start(
            bT[:], txtT[0:P, :].rearrange("p (c n) -> p c n", c=dc_n)
        )
        bT_tiles.append(bT)

        nat_splits = [(0, 1), (1, mc_n // 2), (mc_n // 2, mc_n)]
        for lo, hi in nat_splits:
            nc.sync.dma_start(
                a_nat[:, lo:hi, :],
                img[:, lo * d:hi * d].rearrange("p (m k) -> p m k", m=hi - lo),
            )

        for g in (1, 2):
            bT = sb.tile([P, dc_n, gw], fp8, name="bT", tag="bT", bufs=3)
            nc.sync.dma_start(
                bT[:],
                txtT[g * P:(g + 1) * P, :].rearrange("p (c n) -> p c n", c=dc_n),
            )
            bT_tiles.append(bT)

        hn = mc_n // 2
        for lo, hi in ((0, hn), (hn, mc_n)):
            nc.sync.dma_start(
                b_nat[:, lo:hi, :],
                txtn[:, lo * d:hi * d].rearrange("p (m k) -> p m k", m=hi - lo),
            )

        for g in range(3, ng):
            bT = sb.tile([P, dc_n, gw], fp8, name="bT", tag="bT", bufs=3)
            nc.sync.dma_start(
                bT[:],
                txtT[g * P:(g + 1) * P, :].rearrange("p (c n) -> p c n", c=dc_n),
            )
            bT_tiles.append(bT)

        # image norms on DVE (fused square + free-dim accum)
        norms2_a = sb.tile([P, mc_n], f32, name="norms2_a")
        for m in range(mc_n):
            sqa = sb.tile([P, d], bf16, name="sqa", tag="sqa", bufs=2)
            nc.vector.scalar_tensor_tensor(
                out=sqa[:], in0=a_nat[:, m, :], scalar=1.0, in1=a_nat[:, m, :],
                op0=OP.mult, op1=OP.mult, accum_out=norms2_a[:, m:m + 1],
            )
        ln_a = sb.tile([P, mc_n], f32, name="ln_a")
        ra_act = sb.tile([P, mc_n], f32, name="ra_act")

        d_nat = sb.tile([P, mc_n], f32, name="d_nat")
        row_acc = sb.tile([P, ng * mc_n], f32, name="row_acc")

        # ---------------- main loop over column blocks -----------------------
        for g in range(ng):
            bT = bT_tiles[g]
            col_acc = sb.tile([P, gw], bf16, name="col_acc", tag="col", bufs=2)
            for m in range(mc_n):
                mm = ps.tile([P, gw], f32, name="mm", tag="mm", bufs=3)
                for n2 in range(n2_n):
                    for pc in range(pc_n):
                        nc.tensor.matmul(
                            mm[:, n2 * ns:(n2 + 1) * ns],
                            aT[:, 2 * pc:2 * pc + 2, m * P:(m + 1) * P],
                            bT[:, 2 * pc:2 * pc + 2, n2 * ns:(n2 + 1) * ns],
                            start=(pc == 0), stop=(pc == pc_n - 1),
                            perf_mode=PM,
                        )
                if g == 0:
                    # ra(m) interleaved into ACT program order just before
                    # its first use, so exp(g0, m0) isn't queued behind
                    # stats for later chunks.
                    nc.scalar.activation(
                        ln_a[:, m:m + 1], norms2_a[:, m:m + 1], AF.Ln
                    )
                    nc.scalar.activation(
                        ra_act[:, m:m + 1], ln_a[:, m:m + 1], AF.Exp,
                        scale=-0.5, bias=ln_invt[:],
                    )
                exp_t = sb.tile([P, gw], bf16, name="exp_t", tag="exp", bufs=4)
                slot = g * mc_n + m
                nc.scalar.activation(
                    exp_t[:], mm[:], AF.Exp,
                    scale=ra_act[:, m:m + 1],
                    accum_out=row_acc[:, slot:slot + 1],
                )
                if m == 0:
                    nc.vector.tensor_copy(col_acc[:], exp_t[:])
                else:
                    nc.vector.tensor_add(col_acc[:], col_acc[:], exp_t[:])

            # diag dot products spread across mid blocks (DVE slack, after
            # the late-ordered b_nat halves have landed)
            for mm_d in _diag_sched(g, ng, mc_n):
                prod = sb.tile([P, d], bf16, name="prod", tag="prod", bufs=2)
                nc.vector.scalar_tensor_tensor(
                    out=prod[:], in0=a_nat[:, mm_d, :], scalar=1.0,
                    in1=b_nat[:, mm_d, :],
                    op0=OP.mult, op1=OP.mult,
                    accum_out=d_nat[:, mm_d:mm_d + 1],
                )

            # column partition-reduce, staged through SBUF on DVE for the
            # ReduceScatter input
            cs_row = sb.tile([1, gw], f32, name="cs_row", tag="cs_row", bufs=2)
            for n2 in range(n2_n):
                cs = ps.tile([1, ns], f32, name="cs", tag="cs", bufs=1)
                nc.tensor.matmul(
                    cs[:], ones_bf[:], col_acc[:, n2 * ns:(n2 + 1) * ns],
                    start=True, stop=True,
                )
                nc.vector.tensor_copy(cs_row[0:1, n2 * ns:(n2 + 1) * ns], cs[:])
            nc.gpsimd.dma_start(
                cc_rs_in[0:1, g * gw:(g + 1) * gw], cs_row[:]
            )

        dterm = sb.tile([P, mc_n], f32, name="dterm")
        nc.vector.tensor_mul(dterm[:], d_nat[:], ra_act[:])

        # ---------------- epilogue -------------------------------------------
        if collectives:
            nc.gpsimd.collective_compute(
                "ReduceScatter",
                OP.add,
                replica_groups=rg,
                ins=[cc_rs_in[:].opt()],
                outs=[cc_rs_out[:].opt()],
            )
        else:
            nc.gpsimd.dma_start(cc_rs_out[:], cc_rs_in[0:1, 0:m_loc])

        # my column shard's summed exp: [P, mc_n] (element order irrelevant)
        scol = sb.tile([P, mc_n], f32, name="scol")
        nc.gpsimd.dma_start(
            scol[:], cc_rs_out[0:1, :].rearrange("o (p f) -> p (o f)", p=P)
        )
        lsc = sb.tile([P, mc_n], f32, name="lsc")
        nc.scalar.activation(lsc[:], scol[:], AF.Ln)

        # total row sums: sum slots over g for each m
        srow = sb.tile([P, mc_n], f32, name="srow")
        nc.vector.tensor_reduce(
            srow[:],
            row_acc[:].rearrange("p (g m) -> p m g", g=ng),
            axis=X,
            op=OP.add,
        )
        lsr = sb.tile([P, mc_n], f32, name="lsr")
        nc.scalar.activation(lsr[:], srow[:], AF.Ln)

        # per-partition combine: F = 0.5*(sum lsr + sum lsc) - sum dterm
        s1 = sb.tile([P, 1], f32, name="s1")
        nc.vector.tensor_reduce(s1[:], lsr[:], axis=X, op=OP.add)
        s2 = sb.tile([P, 1], f32, name="s2")
        nc.vector.tensor_reduce(s2[:], lsc[:], axis=X, op=OP.add)
        s3 = sb.tile([P, 1], f32, name="s3")
        nc.vector.tensor_reduce(s3[:], dterm[:], axis=X, op=OP.add)
        tsum = sb.tile([P, 1], f32, name="tsum")
        nc.vector.tensor_add(tsum[:], s1[:], s2[:])
        fvec = sb.tile([P, 1], f32, name="fvec")
        nc.vector.scalar_tensor_tensor(
            out=fvec[:], in0=tsum[:], scalar=0.5, in1=s3[:],
            op0=OP.mult, op1=OP.subtract,
        )

        # partition sum -> scalar partial (scaled by 1/N)
        loss_ps = ps.tile([1, 1], f32, name="loss_ps", tag="cs", bufs=1)
        nc.tensor.matmul(loss_ps[:], ones_f32[:], fvec[:], start=True, stop=True)
        out_sb = sb.tile([1, 1], f32, name="out_sb")
        nc.scalar.mul(out_sb[:], loss_ps[:], 1.0 / n_global)
        nc.gpsimd.dma_start(out_d[0:1, 0:1], out_sb[:])

    nc.compile()
    return nc


def _interleave_weights(wT: np.ndarray) -> np.ndarray:
    """Host-side weight interleave for DoubleRowSwInterleave.

    wT: logical transposed weights [K, M] (fp8).  For each instruction
    slice (k-pair block pc, 128-col chunk mb), the hw expects byte
    fw = ci*128 + mi (ci = k-chunk within pair, mi = col within chunk) to
    hold W_s[p, 127 - jr] where s = fw % 2, jr = fw // 2.
    """
    K, M = wT.shape
    out = np.empty_like(wT)
    fw = np.arange(256)
    ci, mi = fw // 128, fw % 128
    s, jr = fw % 2, fw // 2
    j = 127 - jr
    for pc in range(K // 256):
        blk = wT[pc * 256:(pc + 1) * 256].reshape(2, 128, M // 128, 128)
        # dest [ci, p, mb, mi] = blk[s(fw), p, mb, j(fw)] with fw=ci*128+mi
        dest = blk[s, :, :, j]            # [256(fw), p, mb]
        dest = dest.transpose(1, 2, 0)    # [p, mb, fw]
        dest = dest.reshape(128, M // 128, 2, 128)  # [p, mb, ci, mi]
        out[pc * 256:(pc + 1) * 256] = (
            dest.transpose(2, 0, 1, 3).reshape(256, M)
        )
    return out


def _nat_tiled(x: np.ndarray) -> np.ndarray:
    """[m_loc, d] natural array -> [P, mc_n*d] tile layout [p, m*d+k]."""
    m_loc, d = x.shape
    return np.ascontiguousarray(
        x.reshape(m_loc // P, P, d).transpose(1, 0, 2).reshape(P, -1)
    )


def _kT_tiled(xT: np.ndarray) -> np.ndarray:
    """[K, cols] transposed array -> [P, (K//P)*cols] layout [p, c*cols+n]."""
    K, cols = xT.shape
    return np.ascontiguousarray(
        xT.reshape(K // P, P, cols).transpose(1, 0, 2).reshape(P, -1)
    )


def make_in_maps(image_embeddings: np.ndarray, text_embeddings: np.ndarray):
    n_global, d = image_embeddings.shape
    m_loc = n_global // W
    fp8 = ml_dtypes.float8_e4m3

    tn = text_embeddings / np.maximum(
        np.linalg.norm(text_embeddings, axis=1, keepdims=True), 1e-12
    )
    tn16 = (tn * TXT_PRESCALE).astype(np.float32)
    txtT_q = np.ascontiguousarray(tn16.T).astype(fp8)
    # txtT block-tiled: rows g*128+p, cols c*gw+n  (gw = m_loc)
    txtT_t = np.concatenate(
        [_kT_tiled(txtT_q[:, g * m_loc:(g + 1) * m_loc]) for g in range(W)],
        axis=0,
    )

    maps = []
    for k in range(W):
        sl = slice(k * m_loc, (k + 1) * m_loc)
        aT_q = np.ascontiguousarray(
            image_embeddings[sl].T.astype(np.float32)
        ).astype(fp8)
        maps.append({
            "img": _nat_tiled(image_embeddings[sl].astype(ml_dtypes.bfloat16)),
            "txtn": _nat_tiled(tn16[sl].astype(ml_dtypes.bfloat16)),
            "imgT": _kT_tiled(_interleave_weights(aT_q)),
            "txtT": txtT_t,
        })
    return maps


def kernel(image_embeddings: np.ndarray, text_embeddings: np.ndarray) -> np.ndarray:
    from concourse.bass_utils import run_bass_kernel_spmd

    n_global, d = image_embeddings.shape
    key = (n_global, d)
    if key not in _CACHE:
        _CACHE[key] = build_bass(n_global, d)
    nc = _CACHE[key]

    in_maps = make_in_maps(
        np.asarray(image_embeddings, np.float32),
        np.asarray(text_embeddings, np.float32),
    )
    res = run_bass_kernel_spmd(nc, in_maps, core_ids=list(range(W)))
    total = sum(float(r["partial"][0, 0]) for r in res.results)
    return np.asarray(total, dtype=np.float32)


# revision 16
# speedup vs baseline: 1.1074x; 1.0052x over previous
"""CLIP contrastive loss (nn_ClipLoss) on 8 Trainium2 NeuronCores.

Strategy (row-sharded data parallel, fp8 DoubleRow matmul):
  - Each core k holds its row shard of the image embeddings plus the full
    normalized text matrix transposed (txtT), per the sharding hint: the
    "all-gathered normalized text embeddings" are materialized host-side
    (full_io staging), normalized in f32 and quantized to fp8 e4m3.  Text
    values are pre-scaled by 16 to clear the e4m3 subnormal range; the 1/16
    rides the per-row exp scale.
  - Logits block per core: [1024, 8192] = aT.T @ bT computed in fp8 with
    MatmulPerfMode.DoubleRowSwInterleave: each instruction contracts TWO
    K=128 slices at 0.5 cycles/row (4x bf16 MACs/cycle).  The hw-native
    weight layout (A/B pairs interleaved per column, columns reversed) is
    produced on the host for the stationary image tiles; the moving text
    tiles use the plain [p, 2, n] access pattern.
  - Image-side normalization stays on device: per-row-chunk squared norms
    via fused DVE scalar_tensor_tensor (free-dim accum), then
    ra = 1/(16*T*||a||) via ACT Ln/Exp.  ra rides the ACT Exp `scale`
    per-partition operand so the matmul never waits on a norm chain.
  - exp on ACT with fused row-sum (accum_out).  Column partial sums
    accumulate on DVE in bf16 (2x mode); per-block partition reduction via
    ones-matmul into a dedicated PSUM bank, DMA'd straight to the
    ReduceScatter staging buffer.
  - One 32KB ReduceScatter hands each core its own column shard; each core
    emits one fp32 partial; host sums 8 partials.
"""

import math

import numpy as np
import ml_dtypes

N_FULL = 8192
D_FULL = 1024
W = 8
P = 128
NS = 512
TEMP = 0.07
TXT_PRESCALE = 16.0
LN_SCALE = math.log(1.0 / (TXT_PRESCALE * TEMP))

_CACHE: dict = {}


def _diag_sched(g: int, ng: int, mc_n: int) -> list:
    """Row chunks whose diagonal dot product runs at the end of block g:
    spread chunks over blocks [3, ng-2], two per block for the defaults."""
    first, last = 3, ng - 2
    nblk = max(last - first + 1, 1)
    per = -(-mc_n // nblk)
    if g < first or g > last:
        return []
    lo = (g - first) * per
    return list(range(lo, min(lo + per, mc_n)))


def build_bass(n_global: int = N_FULL, d: int = D_FULL, collectives: bool = True):
    """Build the SPMD bass program (identical on all cores).

    collectives=False replaces the ReduceScatter with a local DMA stand-in
    (for single-core TimelineSim cost modeling only — numerically wrong
    across cores, but dependency/traffic equivalent on one core).
    """
    from contextlib import ExitStack

    import concourse.mybir as mybir
    import concourse.tile as tile
    from concourse import bacc

    f32 = mybir.dt.float32
    bf16 = mybir.dt.bfloat16
    fp8 = mybir.dt.float8e4
    AF = mybir.ActivationFunctionType
    OP = mybir.AluOpType
    X = mybir.AxisListType.X
    PM = mybir.MatmulPerfMode.DoubleRowSwInterleave

    m_loc = n_global // W          # rows per core
    mc_n = m_loc // P              # image row chunks per core
    dc_n = d // P                  # K chunks
    pc_n = dc_n // 2               # K pair-chunks (DoubleRow)
    gw = m_loc                     # column-block width (= shard width)
    ng = n_global // gw            # column blocks
    ns = min(NS, gw)               # psum slice width (one bank)
    n2_n = gw // ns

    import concourse.bacc as bacc_mod

    if not getattr(bacc_mod, "_clip_act_tables_patched", False):
        _orig_tabs = bacc_mod.get_activation_tables

        def _one_set_tables(module_arch):
            tabs = dict(_orig_tabs(module_arch))
            full_name = "natural_log_exp_and_others"
            if full_name in tabs:
                ours = {AF.Ln, AF.Exp, AF.Copy, AF.Identity, AF.Square}
                for name in tabs:
                    if name != full_name:
                        tabs[name] = set(tabs[name]) - ours
            return tabs

        bacc_mod.get_activation_tables = _one_set_tables
        bacc_mod._clip_act_tables_patched = True

    # All inputs are staged host-side in tile layout so every DMA moves
    # contiguous 8KB-per-partition lines (128 descriptors per tile):
    #   img  [P, mc_n*d]  : [p, m*d + k]    = image[m*128+p, k]       (bf16)
    #   txtn [P, mc_n*d]  : same layout for normalized*16 text        (bf16)
    #   imgT [P, dc_n*m]  : [p, c*m_loc+mi] = interleave(imgT)[c*128+p, mi]
    #   txtT [ng*P, dc_n*gw] : [g*128+p, c*gw+n] = txtT_n[c*128+p, g*gw+n]
    nc = bacc.Bacc("TRN2", target_bir_lowering=False, num_devices=W)
    img = nc.dram_tensor("img", [P, mc_n * d], fp8, kind="ExternalInput")
    txtn = nc.dram_tensor("txtn", [P, mc_n * d], fp8, kind="ExternalInput")
    imgT = nc.dram_tensor("imgT", [P, dc_n * m_loc], fp8, kind="ExternalInput")
    txtT = nc.dram_tensor("txtT", [ng * P, dc_n * gw], fp8, kind="ExternalInput")
    out_d = nc.dram_tensor("partial", [1, 1], f32, kind="ExternalOutput")
    rg = [list(range(W))]

    with tile.TileContext(nc) as tc, ExitStack() as ctx:
        sb = ctx.enter_context(tc.tile_pool(name="sb", bufs=1))
        ps = ctx.enter_context(tc.tile_pool(name="ps", bufs=1, space="PSUM"))
        dram = ctx.enter_context(tc.tile_pool(name="dram", bufs=1, space="DRAM"))

        # constants
        ones_bf = sb.tile([P, 1], bf16, name="ones_bf")
        nc.gpsimd.memset(ones_bf[:], 1.0)
        ones_f32 = sb.tile([P, 1], f32, name="ones_f32")
        nc.gpsimd.memset(ones_f32[:], 1.0)
        ln_invt = sb.tile([P, 1], f32, name="ln_invt")
        nc.gpsimd.memset(ln_invt[:], LN_SCALE)

        # collective DRAM buffers
        cc_rs_in = dram.tile([1, n_global], f32, name="cc_rs_in")
        cc_rs_out = dram.tile([1, m_loc], f32, name="cc_rs_out")

        # ---------------- prologue ------------------------------------------
        # Stationary image tiles (host-interleaved fp8) on the gpsimd queue;
        # the first moving text block rides the SP queue in parallel.
        # The sim's DMA bus is a single serialized 341GB/s device, so arrival
        # order == dispatch order matters more than queue choice.  Everything
        # ordering-critical rides the SP queue in exactly the order of need:
        #   bT0, a_nat (3 chunks), bT1, bT2, b_nat (2 halves), bT3..bT7.
        # aT goes on gpsimd so its dispatch precedes bT0 on the bus.
        # ACT gets NO DMA dispatches: a DMACopy holds its SEQ until the
        # transfer completes, which would gate every exp behind it.
        aT = sb.tile([P, dc_n, m_loc], fp8, name="aT")
        nc.gpsimd.dma_start(
            aT[:], imgT[:].rearrange("p (c m) -> p c m", c=dc_n)
        )

        a_nat = sb.tile([P, mc_n, d], fp8, name="a_nat")
        b_nat = sb.tile([P, mc_n, d], fp8, name="b_nat")

        def a_nat_load(lo, hi):
            nc.sync.dma_start(
                a_nat[:, lo:hi, :],
                img[:, lo * d:hi * d].rearrange("p (m k) -> p m k", m=hi - lo),
            )

        bT_tiles = []
        a_nat_load(0, 2)
        bT = sb.tile([P, dc_n, gw], fp8, name="bT", tag="bT", bufs=3)
        nc.sync.dma_start(
            bT[:], txtT[0:P, :].rearrange("p (c n) -> p c n", c=dc_n)
        )
        bT_tiles.append(bT)
        a_nat_load(2, mc_n // 2)
        a_nat_load(mc_n // 2, mc_n)

        for g in (1, 2):
            bT = sb.tile([P, dc_n, gw], fp8, name="bT", tag="bT", bufs=3)
            nc.sync.dma_start(
                bT[:],
                txtT[g * P:(g + 1) * P, :].rearrange("p (c n) -> p c n", c=dc_n),
            )
            bT_tiles.append(bT)

        hn = mc_n // 2
        for lo, hi in ((0, hn), (hn, mc_n)):
            nc.sync.dma_start(
                b_nat[:, lo:hi, :],
                txtn[:, lo * d:hi * d].rearrange("p (m k) -> p m k", m=hi - lo),
            )

        for g in range(3, ng):
            bT = sb.tile([P, dc_n, gw], fp8, name="bT", tag="bT", bufs=3)
            nc.sync.dma_start(
                bT[:],
                txtT[g * P:(g + 1) * P, :].rearrange("p (c n) -> p c n", c=dc_n),
            )
            bT_tiles.append(bT)

        # image norms on DVE (fused square + free-dim accum)
        norms2_a = sb.tile([P, mc_n], f32, name="norms2_a")
        for m in range(mc_n):
            sqa = sb.tile([P, d], bf16, name="sqa", tag="sqa", bufs=2)
            nc.vector.scalar_tensor_tensor(
                out=sqa[:], in0=a_nat[:, m, :], scalar=1.0, in1=a_nat[:, m, :],
                op0=OP.mult, op1=OP.mult, accum_out=norms2_a[:, m:m + 1],
            )
        ln_a = sb.tile([P, mc_n], f32, name="ln_a")
        ra_act = sb.tile([P, mc_n], f32, name="ra_act")

        d_nat = sb.tile([P, mc_n], f32, name="d_nat")
        row_acc = sb.tile([P, ng * mc_n], f32, name="row_acc")

        # ---------------- main loop over column blocks -----------------------
        for g in range(ng):
            bT = bT_tiles[g]
            col_acc = sb.tile([P, gw], bf16, name="col_acc", tag="col", bufs=2)
            for m in range(mc_n):
                mm = ps.tile([P, gw], f32, name="mm", tag="mm", bufs=3)
                for n2 in range(n2_n):
                    for pc in range(pc_n):
                        nc.tensor.matmul(
                            mm[:, n2 * ns:(n2 + 1) * ns],
                            aT[:, 2 * pc:2 * pc + 2, m * P:(m + 1) * P],
                            bT[:, 2 * pc:2 * pc + 2, n2 * ns:(n2 + 1) * ns],
                            start=(pc == 0), stop=(pc == pc_n - 1),
                            perf_mode=PM,
                        )
                if g == 0:
                    # ra(m) interleaved into ACT program order just before
                    # its first use, so exp(g0, m0) isn't queued behind
                    # stats for later chunks.
                    nc.scalar.activation(
                        ln_a[:, m:m + 1], norms2_a[:, m:m + 1], AF.Ln
                    )
                    nc.scalar.activation(
                        ra_act[:, m:m + 1], ln_a[:, m:m + 1], AF.Exp,
                        scale=-0.5, bias=ln_invt[:],
                    )
                exp_t = sb.tile([P, gw], bf16, name="exp_t", tag="exp", bufs=4)
                slot = g * mc_n + m
                nc.scalar.activation(
                    exp_t[:], mm[:], AF.Exp,
                    scale=ra_act[:, m:m + 1],
                    accum_out=row_acc[:, slot:slot + 1],
                )
                if m == 0:
                    nc.vector.tensor_copy(col_acc[:], exp_t[:])
                else:
                    nc.vector.tensor_add(col_acc[:], col_acc[:], exp_t[:])

            # diag dot products spread across mid blocks (DVE slack, after
            # the late-ordered b_nat halves have landed)
            for mm_d in _diag_sched(g, ng, mc_n):
                prod = sb.tile([P, d], bf16, name="prod", tag="prod", bufs=2)
                nc.vector.scalar_tensor_tensor(
                    out=prod[:], in0=a_nat[:, mm_d, :], scalar=1.0,
                    in1=b_nat[:, mm_d, :],
                    op0=OP.mult, op1=OP.mult,
                    accum_out=d_nat[:, mm_d:mm_d + 1],
                )

            # column partition-reduce, staged through SBUF on DVE for the
            # ReduceScatter input
            cs_row = sb.tile([1, gw], f32, name="cs_row", tag="cs_row", bufs=2)
            for n2 in range(n2_n):
                cs = ps.tile([1, ns], f32, name="cs", tag="cs", bufs=1)
                nc.tensor.matmul(
                    cs[:], ones_bf[:], col_acc[:, n2 * ns:(n2 + 1) * ns],
                    start=True, stop=True,
                )
                nc.vector.tensor_copy(cs_row[0:1, n2 * ns:(n2 + 1) * ns], cs[:])
            nc.gpsimd.dma_start(
                cc_rs_in[0:1, g * gw:(g + 1) * gw], cs_row[:]
            )

        dterm = sb.tile([P, mc_n], f32, name="dterm")
        nc.vector.tensor_mul(dterm[:], d_nat[:], ra_act[:])

        # ---------------- epilogue -------------------------------------------
        if collectives:
            nc.gpsimd.collective_compute(
                "ReduceScatter",
                OP.add,
                replica_groups=rg,
                ins=[cc_rs_in[:].opt()],
                outs=[cc_rs_out[:].opt()],
            )
        else:
            nc.gpsimd.dma_start(cc_rs_out[:], cc_rs_in[0:1, 0:m_loc])

        # my column shard's summed exp: [P, mc_n] (element order irrelevant)
        scol = sb.tile([P, mc_n], f32, name="scol")
        nc.gpsimd.dma_start(
            scol[:], cc_rs_out[0:1, :].rearrange("o (p f) -> p (o f)", p=P)
        )
        lsc = sb.tile([P, mc_n], f32, name="lsc")
        nc.scalar.activation(lsc[:], scol[:], AF.Ln)

        # total row sums: sum slots over g for each m
        srow = sb.tile([P, mc_n], f32, name="srow")
        nc.vector.tensor_reduce(
            srow[:],
            row_acc[:].rearrange("p (g m) -> p m g", g=ng),
            axis=X,
            op=OP.add,
        )
        lsr = sb.tile([P, mc_n], f32, name="lsr")
        nc.scalar.activation(lsr[:], srow[:], AF.Ln)

        # per-partition combine: F = 0.5*(sum lsr + sum lsc) - sum dterm
        s1 = sb.tile([P, 1], f32, name="s1")
        nc.vector.tensor_reduce(s1[:], lsr[:], axis=X, op=OP.add)
        s2 = sb.tile([P, 1], f32, name="s2")
        nc.vector.tensor_reduce(s2[:], lsc[:], axis=X, op=OP.add)
        s3 = sb.tile([P, 1], f32, name="s3")
        nc.vector.tensor_reduce(s3[:], dterm[:], axis=X, op=OP.add)
        tsum = sb.tile([P, 1], f32, name="tsum")
        nc.vector.tensor_add(tsum[:], s1[:], s2[:])
        fvec = sb.tile([P, 1], f32, name="fvec")
        nc.vector.scalar_tensor_tensor(
            out=fvec[:], in0=tsum[:], scalar=0.5, in1=s3[:],
            op0=OP.mult, op1=OP.subtract,
        )

        # partition sum -> scalar partial (scaled by 1/N)
        loss_ps = ps.tile([1, 1], f32, name="loss_ps", tag="cs", bufs=1)
        nc.tensor.matmul(loss_ps[:], ones_f32[:], fvec[:], start=True, stop=True)
        out_sb = sb.tile([1, 1], f32, name="out_sb")
        nc.scalar.mul(out_sb[:], loss_ps[:], 1.0 / n_global)
        nc.gpsimd.dma_start(out_d[0:1, 0:1], out_sb[:])

    nc.compile()
    return nc


def _interleave_weights(wT: np.ndarray) -> np.ndarray:
    """Host-side weight interleave for DoubleRowSwInterleave.

    wT: logical transposed weights [K, M] (fp8).  For each instruction
    slice (k-pair block pc, 128-col chunk mb), the hw expects byte
    fw = ci*128 + mi (ci = k-chunk within pair, mi = col within chunk) to
    hold W_s[p, 127 - jr] where s = fw % 2, jr = fw // 2.
    """
    K, M = wT.shape
    out = np.empty_like(wT)
    fw = np.arange(256)
    ci, mi = fw // 128, fw % 128
    s, jr = fw % 2, fw // 2
    j = 127 - jr
    for pc in range(K // 256):
        blk = wT[pc * 256:(pc + 1) * 256].reshape(2, 128, M // 128, 128)
        # dest [ci, p, mb, mi] = blk[s(fw), p, mb, j(fw)] with fw=ci*128+mi
        dest = blk[s, :, :, j]            # [256(fw), p, mb]
        dest = dest.transpose(1, 2, 0)    # [p, mb, fw]
        dest = dest.reshape(128, M // 128, 2, 128)  # [p, mb, ci, mi]
        out[pc * 256:(pc + 1) * 256] = (
            dest.transpose(2, 0, 1, 3).reshape(256, M)
        )
    return out


def _nat_tiled(x: np.ndarray) -> np.ndarray:
    """[m_loc, d] natural array -> [P, mc_n*d] tile layout [p, m*d+k]."""
    m_loc, d = x.shape
    return np.ascontiguousarray(
        x.reshape(m_loc // P, P, d).transpose(1, 0, 2).reshape(P, -1)
    )


def _kT_tiled(xT: np.ndarray) -> np.ndarray:
    """[K, cols] transposed array -> [P, (K//P)*cols] layout [p, c*cols+n]."""
    K, cols = xT.shape
    return np.ascontiguousarray(
        xT.reshape(K // P, P, cols).transpose(1, 0, 2).reshape(P, -1)
    )


def make_in_maps(image_embeddings: np.ndarray, text_embeddings: np.ndarray):
    n_global, d = image_embeddings.shape
    m_loc = n_global // W
    fp8 = ml_dtypes.float8_e4m3

    tn = text_embeddings / np.maximum(
        np.linalg.norm(text_embeddings, axis=1, keepdims=True), 1e-12
    )
    tn16 = (tn * TXT_PRESCALE).astype(np.float32)
    txtT_q = np.ascontiguousarray(tn16.T).astype(fp8)
    # txtT block-tiled: rows g*128+p, cols c*gw+n  (gw = m_loc)
    txtT_t = np.concatenate(
        [_kT_tiled(txtT_q[:, g * m_loc:(g + 1) * m_loc]) for g in range(W)],
        axis=0,
    )

    maps = []
    for k in range(W):
        sl = slice(k * m_loc, (k + 1) * m_loc)
        aT_q = np.ascontiguousarray(
            image_embeddings[sl].T.astype(np.float32)
        ).astype(fp8)
        maps.append({
            "img": _nat_tiled(image_embeddings[sl].astype(fp8)),
            "txtn": _nat_tiled(tn16[sl].astype(fp8)),
            "imgT": _kT_tiled(_interleave_weights(aT_q)),
            "txtT": txtT_t,
        })
    return maps


def kernel(image_embeddings: np.ndarray, text_embeddings: np.ndarray) -> np.ndarray:
    from concourse.bass_utils import run_bass_kernel_spmd

    n_global, d = image_embeddings.shape
    key = (n_global, d)
    if key not in _CACHE:
        _CACHE[key] = build_bass(n_global, d)
    nc = _CACHE[key]

    in_maps = make_in_maps(
        np.asarray(image_embeddings, np.float32),
        np.asarray(text_embeddings, np.float32),
    )
    res = run_bass_kernel_spmd(nc, in_maps, core_ids=list(range(W)))
    total = sum(float(r["partial"][0, 0]) for r in res.results)
    return np.asarray(total, dtype=np.float32)


# revision 22
# speedup vs baseline: 1.1539x; 1.0420x over previous
"""CLIP contrastive loss (nn_ClipLoss) on 8 Trainium2 NeuronCores.

Strategy (row-sharded data parallel, fp8 DoubleRow matmul):
  - Each core k holds its row shard of the image embeddings plus the full
    normalized text matrix transposed (txtT), per the sharding hint: the
    "all-gathered normalized text embeddings" are materialized host-side
    (full_io staging), normalized in f32 and quantized to fp8 e4m3.  Text
    values are pre-scaled by 16 to clear the e4m3 subnormal range; the 1/16
    rides the per-row exp scale.
  - Logits block per core: [1024, 8192] = aT.T @ bT computed in fp8 with
    MatmulPerfMode.DoubleRowSwInterleave: each instruction contracts TWO
    K=128 slices at 0.5 cycles/row (4x bf16 MACs/cycle).  The hw-native
    weight layout (A/B pairs interleaved per column, columns reversed) is
    produced on the host for the stationary image tiles; the moving text
    tiles use the plain [p, 2, n] access pattern.
  - Image-side normalization stays on device: per-row-chunk squared norms
    via fused DVE scalar_tensor_tensor (free-dim accum), then
    ra = 1/(16*T*||a||) via ACT Ln/Exp.  ra rides the ACT Exp `scale`
    per-partition operand so the matmul never waits on a norm chain.
  - exp on ACT with fused row-sum (accum_out).  Column partial sums
    accumulate on DVE in bf16 (2x mode); per-block partition reduction via
    ones-matmul into a dedicated PSUM bank, DMA'd straight to the
    ReduceScatter staging buffer.
  - One 32KB ReduceScatter hands each core its own column shard; each core
    emits one fp32 partial; host sums 8 partials.
"""

import math

import numpy as np
import ml_dtypes

N_FULL = 8192
D_FULL = 1024
W = 8
P = 128
NS = 512
TEMP = 0.07
TXT_PRESCALE = 16.0
LN_SCALE = math.log(1.0 / (TXT_PRESCALE * TEMP))

_CACHE: dict = {}


def _diag_sched(g: int, ng: int, mc_n: int) -> list:
    """Row chunks whose diagonal dot product runs at the end of block g:
    spread chunks over blocks [3, ng-2], two per block for the defaults."""
    first, last = 3, ng - 2
    nblk = max(last - first + 1, 1)
    per = -(-mc_n // nblk)
    if g < first or g > last:
        return []
    lo = (g - first) * per
    return list(range(lo, min(lo + per, mc_n)))


def build_bass(n_global: int = N_FULL, d: int = D_FULL, collectives: bool = True):
    """Build the SPMD bass program (identical on all cores).

    collectives=False replaces the ReduceScatter with a local DMA stand-in
    (for single-core TimelineSim cost modeling only — numerically wrong
    across cores, but dependency/traffic equivalent on one core).
    """
    from contextlib import ExitStack

    import concourse.mybir as mybir
    import concourse.tile as tile
    from concourse import bacc

    f32 = mybir.dt.float32
    bf16 = mybir.dt.bfloat16
    fp8 = mybir.dt.float8e4
    AF = mybir.ActivationFunctionType
    OP = mybir.AluOpType
    X = mybir.AxisListType.X
    PM = mybir.MatmulPerfMode.DoubleRowSwInterleave

    m_loc = n_global // W          # rows per core
    mc_n = m_loc // P              # image row chunks per core
    dc_n = d // P                  # K chunks
    pc_n = dc_n // 2               # K pair-chunks (DoubleRow)
    gw = m_loc                     # column-block width (= shard width)
    ng = n_global // gw            # column blocks
    ns = min(NS, gw)               # psum slice width (one bank)
    n2_n = gw // ns

    import concourse.bacc as bacc_mod

    if not getattr(bacc_mod, "_clip_act_tables_patched", False):
        _orig_tabs = bacc_mod.get_activation_tables

        def _one_set_tables(module_arch):
            tabs = dict(_orig_tabs(module_arch))
            full_name = "natural_log_exp_and_others"
            if full_name in tabs:
                ours = {AF.Ln, AF.Exp, AF.Copy, AF.Identity, AF.Square}
                for name in tabs:
                    if name != full_name:
                        tabs[name] = set(tabs[name]) - ours
            return tabs

        bacc_mod.get_activation_tables = _one_set_tables
        bacc_mod._clip_act_tables_patched = True

    # All inputs are staged host-side in tile layout so every DMA moves
    # contiguous 8KB-per-partition lines (128 descriptors per tile):
    #   img  [P, mc_n*d]  : [p, m*d + k]    = image[m*128+p, k]       (bf16)
    #   txtn [P, mc_n*d]  : same layout for normalized*16 text        (bf16)
    #   imgT [P, dc_n*m]  : [p, c*m_loc+mi] = interleave(imgT)[c*128+p, mi]
    #   txtT [ng*P, dc_n*gw] : [g*128+p, c*gw+n] = txtT_n[c*128+p, g*gw+n]
    nc = bacc.Bacc("TRN2", target_bir_lowering=False, num_devices=W)
    img = nc.dram_tensor("img", [P, mc_n * d], fp8, kind="ExternalInput")
    txtn = nc.dram_tensor("txtn", [P, mc_n * d], fp8, kind="ExternalInput")
    imgT = nc.dram_tensor("imgT", [P, dc_n * m_loc], fp8, kind="ExternalInput")
    txtT = nc.dram_tensor("txtT", [ng * P, dc_n * gw], fp8, kind="ExternalInput")
    out_d = nc.dram_tensor("partial", [1, 1], f32, kind="ExternalOutput")
    rg = [list(range(W))]

    with tile.TileContext(nc) as tc, ExitStack() as ctx:
        sb = ctx.enter_context(tc.tile_pool(name="sb", bufs=1))
        ps = ctx.enter_context(tc.tile_pool(name="ps", bufs=1, space="PSUM"))
        dram = ctx.enter_context(tc.tile_pool(name="dram", bufs=1, space="DRAM"))

        # constants
        ones_bf = sb.tile([P, 1], bf16, name="ones_bf")
        nc.gpsimd.memset(ones_bf[:], 1.0)
        ones_f32 = sb.tile([P, 1], f32, name="ones_f32")
        nc.gpsimd.memset(ones_f32[:], 1.0)
        ln_invt = sb.tile([P, 1], f32, name="ln_invt")
        nc.gpsimd.memset(ln_invt[:], LN_SCALE)

        # collective DRAM buffers
        cc_rs_in = dram.tile([1, n_global], f32, name="cc_rs_in")
        cc_rs_out = dram.tile([1, m_loc], f32, name="cc_rs_out")

        # ---------------- prologue ------------------------------------------
        # Stationary image tiles (host-interleaved fp8) on the gpsimd queue;
        # the first moving text block rides the SP queue in parallel.
        # The sim's DMA bus is a single serialized 341GB/s device, so arrival
        # order == dispatch order matters more than queue choice.  Everything
        # ordering-critical rides the SP queue in exactly the order of need:
        #   bT0, a_nat (3 chunks), bT1, bT2, b_nat (2 halves), bT3..bT7.
        # aT goes on gpsimd so its dispatch precedes bT0 on the bus.
        # ACT gets NO DMA dispatches: a DMACopy holds its SEQ until the
        # transfer completes, which would gate every exp behind it.
        # aT is m-major on the host ([p, m*d + c*128 + mi]) so each row
        # chunk's stationary weights arrive as one contiguous DMA, in need
        # order, interleaved with the a_nat chunks that feed ra.
        aT = sb.tile([P, mc_n, dc_n, P], fp8, name="aT")
        a_nat = sb.tile([P, mc_n, d], fp8, name="a_nat")
        b_nat = sb.tile([P, mc_n, d], fp8, name="b_nat")

        def a_nat_load(lo, hi):
            nc.sync.dma_start(
                a_nat[:, lo:hi, :],
                img[:, lo * d:hi * d].rearrange("p (m k) -> p m k", m=hi - lo),
            )

        def aT_load(lo, hi):
            nc.sync.dma_start(
                aT[:, lo:hi, :, :],
                imgT[:, lo * d:hi * d].rearrange(
                    "p (m c i) -> p m c i", m=hi - lo, c=dc_n
                ),
            )

        bT_tiles = []
        bT = sb.tile([P, dc_n, gw], fp8, name="bT", tag="bT", bufs=3)
        nc.sync.dma_start(
            bT[:], txtT[0:P, :].rearrange("p (c n) -> p c n", c=dc_n)
        )
        bT_tiles.append(bT)
        aT_load(0, 1)
        a_nat_load(0, 2)
        aT_load(1, 3)
        a_nat_load(2, mc_n // 2)
        aT_load(3, 5)
        a_nat_load(mc_n // 2, mc_n)
        aT_load(5, mc_n)

        for g in (1, 2):
            bT = sb.tile([P, dc_n, gw], fp8, name="bT", tag="bT", bufs=3)
            nc.sync.dma_start(
                bT[:],
                txtT[g * P:(g + 1) * P, :].rearrange("p (c n) -> p c n", c=dc_n),
            )
            bT_tiles.append(bT)

        hn = mc_n // 2
        for lo, hi in ((0, hn), (hn, mc_n)):
            nc.sync.dma_start(
                b_nat[:, lo:hi, :],
                txtn[:, lo * d:hi * d].rearrange("p (m k) -> p m k", m=hi - lo),
            )

        for g in range(3, ng):
            bT = sb.tile([P, dc_n, gw], fp8, name="bT", tag="bT", bufs=3)
            nc.sync.dma_start(
                bT[:],
                txtT[g * P:(g + 1) * P, :].rearrange("p (c n) -> p c n", c=dc_n),
            )
            bT_tiles.append(bT)

        # image norms on DVE (fused square + free-dim accum)
        norms2_a = sb.tile([P, mc_n], f32, name="norms2_a")
        for m in range(mc_n):
            sqa = sb.tile([P, d], bf16, name="sqa", tag="sqa", bufs=2)
            nc.vector.scalar_tensor_tensor(
                out=sqa[:], in0=a_nat[:, m, :], scalar=1.0, in1=a_nat[:, m, :],
                op0=OP.mult, op1=OP.mult, accum_out=norms2_a[:, m:m + 1],
            )
        ln_a = sb.tile([P, mc_n], f32, name="ln_a")
        ra_act = sb.tile([P, mc_n], f32, name="ra_act")

        d_nat = sb.tile([P, mc_n], f32, name="d_nat")
        row_acc = sb.tile([P, ng * mc_n], f32, name="row_acc")

        # ---------------- main loop over column blocks -----------------------
        for g in range(ng):
            bT = bT_tiles[g]
            col_acc = sb.tile([P, gw], bf16, name="col_acc", tag="col", bufs=2)
            for m in range(mc_n):
                mm = ps.tile([P, gw], f32, name="mm", tag="mm", bufs=3)
                for n2 in range(n2_n):
                    for pc in range(pc_n):
                        nc.tensor.matmul(
                            mm[:, n2 * ns:(n2 + 1) * ns],
                            aT[:, m, 2 * pc:2 * pc + 2, :],
                            bT[:, 2 * pc:2 * pc + 2, n2 * ns:(n2 + 1) * ns],
                            start=(pc == 0), stop=(pc == pc_n - 1),
                            perf_mode=PM,
                        )
                if g == 0:
                    # ra(m) interleaved into ACT program order just before
                    # its first use, so exp(g0, m0) isn't queued behind
                    # stats for later chunks.
                    nc.scalar.activation(
                        ln_a[:, m:m + 1], norms2_a[:, m:m + 1], AF.Ln
                    )
                    nc.scalar.activation(
                        ra_act[:, m:m + 1], ln_a[:, m:m + 1], AF.Exp,
                        scale=-0.5, bias=ln_invt[:],
                    )
                exp_t = sb.tile([P, gw], bf16, name="exp_t", tag="exp", bufs=4)
                slot = g * mc_n + m
                nc.scalar.activation(
                    exp_t[:], mm[:], AF.Exp,
                    scale=ra_act[:, m:m + 1],
                    accum_out=row_acc[:, slot:slot + 1],
                )
                if m == 0:
                    nc.vector.tensor_copy(col_acc[:], exp_t[:])
                else:
                    nc.vector.tensor_add(col_acc[:], col_acc[:], exp_t[:])

            # diag dot products spread across mid blocks (DVE slack, after
            # the late-ordered b_nat halves have landed)
            for mm_d in _diag_sched(g, ng, mc_n):
                prod = sb.tile([P, d], bf16, name="prod", tag="prod", bufs=2)
                nc.vector.scalar_tensor_tensor(
                    out=prod[:], in0=a_nat[:, mm_d, :], scalar=1.0,
                    in1=b_nat[:, mm_d, :],
                    op0=OP.mult, op1=OP.mult,
                    accum_out=d_nat[:, mm_d:mm_d + 1],
                )

            # column partition-reduce, staged through SBUF on DVE for the
            # ReduceScatter input
            cs_row = sb.tile([1, gw], f32, name="cs_row", tag="cs_row", bufs=2)
            for n2 in range(n2_n):
                cs = ps.tile([1, ns], f32, name="cs", tag="cs", bufs=1)
                nc.tensor.matmul(
                    cs[:], ones_bf[:], col_acc[:, n2 * ns:(n2 + 1) * ns],
                    start=True, stop=True,
                )
                nc.vector.tensor_copy(cs_row[0:1, n2 * ns:(n2 + 1) * ns], cs[:])
            nc.gpsimd.dma_start(
                cc_rs_in[0:1, g * gw:(g + 1) * gw], cs_row[:]
            )

            # row-sum partial over blocks 0..ng-2 so only the last block's
            # slots remain on the critical tail
            if g == ng - 2:
                srow_part = sb.tile([P, mc_n], f32, name="srow_part")
                nc.vector.tensor_reduce(
                    srow_part[:],
                    row_acc[:, 0:(ng - 1) * mc_n].rearrange(
                        "p (g m) -> p m g", g=ng - 1
                    ),
                    axis=X,
                    op=OP.add,
                )

        dterm = sb.tile([P, mc_n], f32, name="dterm")
        nc.vector.tensor_mul(dterm[:], d_nat[:], ra_act[:])

        # ---------------- epilogue -------------------------------------------
        if collectives:
            nc.gpsimd.collective_compute(
                "ReduceScatter",
                OP.add,
                replica_groups=rg,
                ins=[cc_rs_in[:].opt()],
                outs=[cc_rs_out[:].opt()],
            )
        else:
            nc.gpsimd.dma_start(cc_rs_out[:], cc_rs_in[0:1, 0:m_loc])

        # my column shard's summed exp: [P, mc_n] (element order irrelevant)
        scol = sb.tile([P, mc_n], f32, name="scol")
        nc.gpsimd.dma_start(
            scol[:], cc_rs_out[0:1, :].rearrange("o (p f) -> p (o f)", p=P)
        )
        lsc = sb.tile([P, mc_n], f32, name="lsc")
        nc.scalar.activation(lsc[:], scol[:], AF.Ln)

        # total row sums: partial (blocks 0..ng-2) + last block's slots
        srow = sb.tile([P, mc_n], f32, name="srow")
        nc.vector.tensor_add(
            srow[:], srow_part[:], row_acc[:, (ng - 1) * mc_n:ng * mc_n]
        )
        lsr = sb.tile([P, mc_n], f32, name="lsr")
        nc.scalar.activation(lsr[:], srow[:], AF.Ln)

        # per-partition combine: F = 0.5*(sum lsr + sum lsc) - sum dterm
        s1 = sb.tile([P, 1], f32, name="s1")
        nc.vector.tensor_reduce(s1[:], lsr[:], axis=X, op=OP.add)
        s2 = sb.tile([P, 1], f32, name="s2")
        nc.vector.tensor_reduce(s2[:], lsc[:], axis=X, op=OP.add)
        s3 = sb.tile([P, 1], f32, name="s3")
        nc.vector.tensor_reduce(s3[:], dterm[:], axis=X, op=OP.add)
        tsum = sb.tile([P, 1], f32, name="tsum")
        nc.vector.tensor_add(tsum[:], s1[:], s2[:])
        fvec = sb.tile([P, 1], f32, name="fvec")
        nc.vector.scalar_tensor_tensor(
            out=fvec[:], in0=tsum[:], scalar=0.5, in1=s3[:],
            op0=OP.mult, op1=OP.subtract,
        )

        # partition sum -> scalar partial (scaled by 1/N)
        loss_ps = ps.tile([1, 1], f32, name="loss_ps", tag="cs", bufs=1)
        nc.tensor.matmul(loss_ps[:], ones_f32[:], fvec[:], start=True, stop=True)
        out_sb = sb.tile([1, 1], f32, name="out_sb")
        nc.scalar.mul(out_sb[:], loss_ps[:], 1.0 / n_global)
        nc.gpsimd.dma_start(out_d[0:1, 0:1], out_sb[:])

    nc.compile()
    return nc


def _interleave_weights(wT: np.ndarray) -> np.ndarray:
    """Host-side weight interleave for DoubleRowSwInterleave.

    wT: logical transposed weights [K, M] (fp8).  For each instruction
    slice (k-pair block pc, 128-col chunk mb), the hw expects byte
    fw = ci*128 + mi (ci = k-chunk within pair, mi = col within chunk) to
    hold W_s[p, 127 - jr] where s = fw % 2, jr = fw // 2.
    """
    K, M = wT.shape
    out = np.empty_like(wT)
    fw = np.arange(256)
    ci, mi = fw // 128, fw % 128
    s, jr = fw % 2, fw // 2
    j = 127 - jr
    for pc in range(K // 256):
        blk = wT[pc * 256:(pc + 1) * 256].reshape(2, 128, M // 128, 128)
        # dest [ci, p, mb, mi] = blk[s(fw), p, mb, j(fw)] with fw=ci*128+mi
        dest = blk[s, :, :, j]            # [256(fw), p, mb]
        dest = dest.transpose(1, 2, 0)    # [p, mb, fw]
        dest = dest.reshape(128, M // 128, 2, 128)  # [p, mb, ci, mi]
        out[pc * 256:(pc + 1) * 256] = (
            dest.transpose(2, 0, 1, 3).reshape(256, M)
        )
    return out


def _nat_tiled(x: np.ndarray) -> np.ndarray:
    """[m_loc, d] natural array -> [P, mc_n*d] tile layout [p, m*d+k]."""
    m_loc, d = x.shape
    return np.ascontiguousarray(
        x.reshape(m_loc // P, P, d).transpose(1, 0, 2).reshape(P, -1)
    )


def _mT_tiled(xT: np.ndarray) -> np.ndarray:
    """[K, M] transposed array -> [P, M//P * K] m-major tile layout
    [p, mb*(K) + c*P + mi] = xT[c*P+p, mb*P+mi]."""
    K, M = xT.shape
    return np.ascontiguousarray(
        xT.reshape(K // P, P, M // P, P).transpose(1, 2, 0, 3).reshape(P, -1)
    )


def _kT_tiled(xT: np.ndarray) -> np.ndarray:
    """[K, cols] transposed array -> [P, (K//P)*cols] layout [p, c*cols+n]."""
    K, cols = xT.shape
    return np.ascontiguousarray(
        xT.reshape(K // P, P, cols).transpose(1, 0, 2).reshape(P, -1)
    )


def make_in_maps(image_embeddings: np.ndarray, text_embeddings: np.ndarray):
    n_global, d = image_embeddings.shape
    m_loc = n_global // W
    fp8 = ml_dtypes.float8_e4m3

    tn = text_embeddings / np.maximum(
        np.linalg.norm(text_embeddings, axis=1, keepdims=True), 1e-12
    )
    tn16 = (tn * TXT_PRESCALE).astype(np.float32)
    txtT_q = np.ascontiguousarray(tn16.T).astype(fp8)
    # txtT block-tiled: rows g*128+p, cols c*gw+n  (gw = m_loc)
    txtT_t = np.concatenate(
        [_kT_tiled(txtT_q[:, g * m_loc:(g + 1) * m_loc]) for g in range(W)],
        axis=0,
    )

    maps = []
    for k in range(W):
        sl = slice(k * m_loc, (k + 1) * m_loc)
        aT_q = np.ascontiguousarray(
            image_embeddings[sl].T.astype(np.float32)
        ).astype(fp8)
        maps.append({
            "img": _nat_tiled(image_embeddings[sl].astype(fp8)),
            "txtn": _nat_tiled(tn16[sl].astype(fp8)),
            "imgT": _mT_tiled(_interleave_weights(aT_q)),
            "txtT": txtT_t,
        })
    return maps


def kernel(image_embeddings: np.ndarray, text_embeddings: np.ndarray) -> np.ndarray:
    from concourse.bass_utils import run_bass_kernel_spmd

    n_global, d = image_embeddings.shape
    key = (n_global, d)
    if key not in _CACHE:
        _CACHE[key] = build_bass(n_global, d)
    nc = _CACHE[key]

    in_maps = make_in_maps(
        np.asarray(image_embeddings, np.float32),
        np.asarray(text_embeddings, np.float32),
    )
    res = run_bass_kernel_spmd(nc, in_maps, core_ids=list(range(W)))
    total = sum(float(r["partial"][0, 0]) for r in res.results)
    return np.asarray(total, dtype=np.float32)


# revision 24
# speedup vs baseline: 1.1540x; 1.0001x over previous
"""CLIP contrastive loss (nn_ClipLoss) on 8 Trainium2 NeuronCores.

Strategy (row-sharded data parallel, fp8 DoubleRow matmul):
  - Each core k holds its row shard of the image embeddings plus the full
    normalized text matrix transposed (txtT), per the sharding hint: the
    "all-gathered normalized text embeddings" are materialized host-side
    (full_io staging), normalized in f32 and quantized to fp8 e4m3.  Text
    values are pre-scaled by 16 to clear the e4m3 subnormal range; the 1/16
    rides the per-row exp scale.
  - Logits block per core: [1024, 8192] = aT.T @ bT computed in fp8 with
    MatmulPerfMode.DoubleRowSwInterleave: each instruction contracts TWO
    K=128 slices at 0.5 cycles/row (4x bf16 MACs/cycle).  The hw-native
    weight layout (A/B pairs interleaved per column, columns reversed) is
    produced on the host for the stationary image tiles; the moving text
    tiles use the plain [p, 2, n] access pattern.
  - Image-side normalization stays on device: per-row-chunk squared norms
    via fused DVE scalar_tensor_tensor (free-dim accum), then
    ra = 1/(16*T*||a||) via ACT Ln/Exp.  ra rides the ACT Exp `scale`
    per-partition operand so the matmul never waits on a norm chain.
  - exp on ACT with fused row-sum (accum_out).  Column partial sums
    accumulate on DVE in bf16 (2x mode); per-block partition reduction via
    ones-matmul into a dedicated PSUM bank, DMA'd straight to the
    ReduceScatter staging buffer.
  - One 32KB ReduceScatter hands each core its own column shard; each core
    emits one fp32 partial; host sums 8 partials.
"""

import math

import numpy as np
import ml_dtypes

N_FULL = 8192
D_FULL = 1024
W = 8
P = 128
NS = 512
TEMP = 0.07
TXT_PRESCALE = 16.0
LN_SCALE = math.log(1.0 / (TXT_PRESCALE * TEMP))

_CACHE: dict = {}


def _diag_sched(g: int, ng: int, mc_n: int) -> list:
    """Row chunks whose diagonal dot product runs at the end of block g:
    spread chunks over blocks [3, ng-2], two per block for the defaults."""
    first, last = 3, ng - 2
    nblk = max(last - first + 1, 1)
    per = -(-mc_n // nblk)
    if g < first or g > last:
        return []
    lo = (g - first) * per
    return list(range(lo, min(lo + per, mc_n)))


def build_bass(n_global: int = N_FULL, d: int = D_FULL, collectives: bool = True):
    """Build the SPMD bass program (identical on all cores).

    collectives=False replaces the ReduceScatter with a local DMA stand-in
    (for single-core TimelineSim cost modeling only — numerically wrong
    across cores, but dependency/traffic equivalent on one core).
    """
    from contextlib import ExitStack

    import concourse.mybir as mybir
    import concourse.tile as tile
    from concourse import bacc

    f32 = mybir.dt.float32
    bf16 = mybir.dt.bfloat16
    fp8 = mybir.dt.float8e4
    AF = mybir.ActivationFunctionType
    OP = mybir.AluOpType
    X = mybir.AxisListType.X
    PM = mybir.MatmulPerfMode.DoubleRowSwInterleave

    m_loc = n_global // W          # rows per core
    mc_n = m_loc // P              # image row chunks per core
    dc_n = d // P                  # K chunks
    pc_n = dc_n // 2               # K pair-chunks (DoubleRow)
    gw = m_loc                     # column-block width (= shard width)
    ng = n_global // gw            # column blocks
    ns = min(NS, gw)               # psum slice width (one bank)
    n2_n = gw // ns

    import concourse.bacc as bacc_mod

    if not getattr(bacc_mod, "_clip_act_tables_patched", False):
        _orig_tabs = bacc_mod.get_activation_tables

        def _one_set_tables(module_arch):
            tabs = dict(_orig_tabs(module_arch))
            full_name = "natural_log_exp_and_others"
            if full_name in tabs:
                ours = {AF.Ln, AF.Exp, AF.Copy, AF.Identity, AF.Square}
                for name in tabs:
                    if name != full_name:
                        tabs[name] = set(tabs[name]) - ours
            return tabs

        bacc_mod.get_activation_tables = _one_set_tables
        bacc_mod._clip_act_tables_patched = True

    # All inputs are staged host-side in tile layout so every DMA moves
    # contiguous 8KB-per-partition lines (128 descriptors per tile):
    #   img  [P, mc_n*d]  : [p, m*d + k]    = image[m*128+p, k]       (bf16)
    #   txtn [P, mc_n*d]  : same layout for normalized*16 text        (bf16)
    #   imgT [P, dc_n*m]  : [p, c*m_loc+mi] = interleave(imgT)[c*128+p, mi]
    #   txtT [ng*P, dc_n*gw] : [g*128+p, c*gw+n] = txtT_n[c*128+p, g*gw+n]
    nc = bacc.Bacc("TRN2", target_bir_lowering=False, num_devices=W)
    img = nc.dram_tensor("img", [P, mc_n * d], fp8, kind="ExternalInput")
    txtn = nc.dram_tensor("txtn", [P, mc_n * d], fp8, kind="ExternalInput")
    imgT = nc.dram_tensor("imgT", [P, dc_n * m_loc], fp8, kind="ExternalInput")
    txtT = nc.dram_tensor("txtT", [ng * P, dc_n * gw], fp8, kind="ExternalInput")
    out_d = nc.dram_tensor("partial", [1, 1], f32, kind="ExternalOutput")
    rg = [list(range(W))]

    with tile.TileContext(nc) as tc, ExitStack() as ctx:
        sb = ctx.enter_context(tc.tile_pool(name="sb", bufs=1))
        ps = ctx.enter_context(tc.tile_pool(name="ps", bufs=1, space="PSUM"))
        dram = ctx.enter_context(tc.tile_pool(name="dram", bufs=1, space="DRAM"))

        # constants
        ones_bf = sb.tile([P, 1], bf16, name="ones_bf")
        nc.gpsimd.memset(ones_bf[:], 1.0)
        ones_f32 = sb.tile([P, 1], f32, name="ones_f32")
        nc.gpsimd.memset(ones_f32[:], 1.0)
        ln_invt = sb.tile([P, 1], f32, name="ln_invt")
        nc.gpsimd.memset(ln_invt[:], LN_SCALE)

        # collective DRAM buffers
        cc_rs_in = dram.tile([1, n_global], f32, name="cc_rs_in")
        cc_rs_out = dram.tile([1, m_loc], f32, name="cc_rs_out")

        # ---------------- prologue ------------------------------------------
        # Stationary image tiles (host-interleaved fp8) on the gpsimd queue;
        # the first moving text block rides the SP queue in parallel.
        # The sim's DMA bus is a single serialized 341GB/s device, so arrival
        # order == dispatch order matters more than queue choice.  Everything
        # ordering-critical rides the SP queue in exactly the order of need:
        #   bT0, a_nat (3 chunks), bT1, bT2, b_nat (2 halves), bT3..bT7.
        # aT goes on gpsimd so its dispatch precedes bT0 on the bus.
        # ACT gets NO DMA dispatches: a DMACopy holds its SEQ until the
        # transfer completes, which would gate every exp behind it.
        # aT is m-major on the host ([p, m*d + c*128 + mi]) so each row
        # chunk's stationary weights arrive as one contiguous DMA, in need
        # order, interleaved with the a_nat chunks that feed ra.
        aT = sb.tile([P, mc_n, dc_n, P], fp8, name="aT")
        a_nat = sb.tile([P, mc_n, d], fp8, name="a_nat")
        b_nat = sb.tile([P, mc_n, d], fp8, name="b_nat")

        def a_nat_load(lo, hi):
            nc.sync.dma_start(
                a_nat[:, lo:hi, :],
                img[:, lo * d:hi * d].rearrange("p (m k) -> p m k", m=hi - lo),
            )

        def aT_load(lo, hi):
            nc.sync.dma_start(
                aT[:, lo:hi, :, :],
                imgT[:, lo * d:hi * d].rearrange(
                    "p (m c i) -> p m c i", m=hi - lo, c=dc_n
                ),
            )

        bT_tiles = []
        bT = sb.tile([P, dc_n, gw], fp8, name="bT", tag="bT", bufs=3)
        nc.sync.dma_start(
            bT[:], txtT[0:P, :].rearrange("p (c n) -> p c n", c=dc_n)
        )
        bT_tiles.append(bT)
        aT_load(0, 2)
        a_nat_load(0, 2)
        aT_load(2, 4)
        a_nat_load(2, mc_n // 2)
        aT_load(4, 6)
        a_nat_load(mc_n // 2, mc_n)
        aT_load(6, mc_n)

        for g in (1, 2):
            bT = sb.tile([P, dc_n, gw], fp8, name="bT", tag="bT", bufs=3)
            nc.sync.dma_start(
                bT[:],
                txtT[g * P:(g + 1) * P, :].rearrange("p (c n) -> p c n", c=dc_n),
            )
            bT_tiles.append(bT)

        hn = mc_n // 2
        for lo, hi in ((0, hn), (hn, mc_n)):
            nc.sync.dma_start(
                b_nat[:, lo:hi, :],
                txtn[:, lo * d:hi * d].rearrange("p (m k) -> p m k", m=hi - lo),
            )

        for g in range(3, ng):
            bT = sb.tile([P, dc_n, gw], fp8, name="bT", tag="bT", bufs=3)
            nc.sync.dma_start(
                bT[:],
                txtT[g * P:(g + 1) * P, :].rearrange("p (c n) -> p c n", c=dc_n),
            )
            bT_tiles.append(bT)

        # image norms on DVE (fused square + free-dim accum)
        norms2_a = sb.tile([P, mc_n], f32, name="norms2_a")
        for m in range(mc_n):
            sqa = sb.tile([P, d], bf16, name="sqa", tag="sqa", bufs=2)
            nc.vector.scalar_tensor_tensor(
                out=sqa[:], in0=a_nat[:, m, :], scalar=1.0, in1=a_nat[:, m, :],
                op0=OP.mult, op1=OP.mult, accum_out=norms2_a[:, m:m + 1],
            )
        ln_a = sb.tile([P, mc_n], f32, name="ln_a")
        ra_act = sb.tile([P, mc_n], f32, name="ra_act")

        d_nat = sb.tile([P, mc_n], f32, name="d_nat")
        row_acc = sb.tile([P, ng * mc_n], f32, name="row_acc")

        # ---------------- main loop over column blocks -----------------------
        for g in range(ng):
            bT = bT_tiles[g]
            col_acc = sb.tile([P, gw], bf16, name="col_acc", tag="col", bufs=2)
            for m in range(mc_n):
                mm = ps.tile([P, gw], f32, name="mm", tag="mm", bufs=3)
                for n2 in range(n2_n):
                    for pc in range(pc_n):
                        nc.tensor.matmul(
                            mm[:, n2 * ns:(n2 + 1) * ns],
                            aT[:, m, 2 * pc:2 * pc + 2, :],
                            bT[:, 2 * pc:2 * pc + 2, n2 * ns:(n2 + 1) * ns],
                            start=(pc == 0), stop=(pc == pc_n - 1),
                            perf_mode=PM,
                        )
                if g == 0:
                    # ra(m) interleaved into ACT program order just before
                    # its first use, so exp(g0, m0) isn't queued behind
                    # stats for later chunks.
                    nc.scalar.activation(
                        ln_a[:, m:m + 1], norms2_a[:, m:m + 1], AF.Ln
                    )
                    nc.scalar.activation(
                        ra_act[:, m:m + 1], ln_a[:, m:m + 1], AF.Exp,
                        scale=-0.5, bias=ln_invt[:],
                    )
                exp_t = sb.tile([P, gw], bf16, name="exp_t", tag="exp", bufs=4)
                slot = g * mc_n + m
                nc.scalar.activation(
                    exp_t[:], mm[:], AF.Exp,
                    scale=ra_act[:, m:m + 1],
                    accum_out=row_acc[:, slot:slot + 1],
                )
                if m == 0:
                    nc.vector.tensor_copy(col_acc[:], exp_t[:])
                else:
                    nc.vector.tensor_add(col_acc[:], col_acc[:], exp_t[:])

            # diag dot products spread across mid blocks (DVE slack, after
            # the late-ordered b_nat halves have landed)
            for mm_d in _diag_sched(g, ng, mc_n):
                prod = sb.tile([P, d], bf16, name="prod", tag="prod", bufs=2)
                nc.vector.scalar_tensor_tensor(
                    out=prod[:], in0=a_nat[:, mm_d, :], scalar=1.0,
                    in1=b_nat[:, mm_d, :],
                    op0=OP.mult, op1=OP.mult,
                    accum_out=d_nat[:, mm_d:mm_d + 1],
                )

            # column partition-reduce, staged through SBUF on DVE for the
            # ReduceScatter input
            cs_row = sb.tile([1, gw], f32, name="cs_row", tag="cs_row", bufs=2)
            for n2 in range(n2_n):
                cs = ps.tile([1, ns], f32, name="cs", tag="cs", bufs=2)
                nc.tensor.matmul(
                    cs[:], ones_bf[:], col_acc[:, n2 * ns:(n2 + 1) * ns],
                    start=True, stop=True,
                )
                nc.vector.tensor_copy(cs_row[0:1, n2 * ns:(n2 + 1) * ns], cs[:])
            nc.gpsimd.dma_start(
                cc_rs_in[0:1, g * gw:(g + 1) * gw], cs_row[:]
            )

            # row-sum partial over blocks 0..ng-2 so only the last block's
            # slots remain on the critical tail
            if g == ng - 2:
                srow_part = sb.tile([P, mc_n], f32, name="srow_part")
                nc.vector.tensor_reduce(
                    srow_part[:],
                    row_acc[:, 0:(ng - 1) * mc_n].rearrange(
                        "p (g m) -> p m g", g=ng - 1
                    ),
                    axis=X,
                    op=OP.add,
                )

        dterm = sb.tile([P, mc_n], f32, name="dterm")
        nc.vector.tensor_mul(dterm[:], d_nat[:], ra_act[:])

        # ---------------- epilogue -------------------------------------------
        if collectives:
            nc.gpsimd.collective_compute(
                "ReduceScatter",
                OP.add,
                replica_groups=rg,
                ins=[cc_rs_in[:].opt()],
                outs=[cc_rs_out[:].opt()],
            )
        else:
            nc.gpsimd.dma_start(cc_rs_out[:], cc_rs_in[0:1, 0:m_loc])

        # my column shard's summed exp: [P, mc_n] (element order irrelevant)
        scol = sb.tile([P, mc_n], f32, name="scol")
        nc.sync.dma_start(
            scol[:], cc_rs_out[0:1, :].rearrange("o (p f) -> p (o f)", p=P)
        )
        lsc = sb.tile([P, mc_n], f32, name="lsc")
        nc.scalar.activation(lsc[:], scol[:], AF.Ln)

        # total row sums: partial (blocks 0..ng-2) + last block's slots
        srow = sb.tile([P, mc_n], f32, name="srow")
        nc.vector.tensor_add(
            srow[:], srow_part[:], row_acc[:, (ng - 1) * mc_n:ng * mc_n]
        )
        lsr = sb.tile([P, mc_n], f32, name="lsr")
        nc.scalar.activation(lsr[:], srow[:], AF.Ln)

        # per-partition combine: F = 0.5*(sum lsr + sum lsc) - sum dterm
        s1 = sb.tile([P, 1], f32, name="s1")
        nc.vector.tensor_reduce(s1[:], lsr[:], axis=X, op=OP.add)
        s2 = sb.tile([P, 1], f32, name="s2")
        nc.vector.tensor_reduce(s2[:], lsc[:], axis=X, op=OP.add)
        s3 = sb.tile([P, 1], f32, name="s3")
        nc.vector.tensor_reduce(s3[:], dterm[:], axis=X, op=OP.add)
        tsum = sb.tile([P, 1], f32, name="tsum")
        nc.vector.tensor_add(tsum[:], s1[:], s2[:])
        fvec = sb.tile([P, 1], f32, name="fvec")
        nc.vector.scalar_tensor_tensor(
            out=fvec[:], in0=tsum[:], scalar=0.5, in1=s3[:],
            op0=OP.mult, op1=OP.subtract,
        )

        # partition sum -> scalar partial (scaled by 1/N)
        loss_ps = ps.tile([1, 1], f32, name="loss_ps", tag="cs", bufs=2)
        nc.tensor.matmul(loss_ps[:], ones_f32[:], fvec[:], start=True, stop=True)
        out_sb = sb.tile([1, 1], f32, name="out_sb")
        nc.scalar.mul(out_sb[:], loss_ps[:], 1.0 / n_global)
        nc.sync.dma_start(out_d[0:1, 0:1], out_sb[:])

    nc.compile()
    return nc


def _interleave_weights(wT: np.ndarray) -> np.ndarray:
    """Host-side weight interleave for DoubleRowSwInterleave.

    wT: logical transposed weights [K, M] (fp8).  For each instruction
    slice (k-pair block pc, 128-col chunk mb), the hw expects byte
    fw = ci*128 + mi (ci = k-chunk within pair, mi = col within chunk) to
    hold W_s[p, 127 - jr] where s = fw % 2, jr = fw // 2.
    """
    K, M = wT.shape
    out = np.empty_like(wT)
    fw = np.arange(256)
    ci, mi = fw // 128, fw % 128
    s, jr = fw % 2, fw // 2
    j = 127 - jr
    for pc in range(K // 256):
        blk = wT[pc * 256:(pc + 1) * 256].reshape(2, 128, M // 128, 128)
        # dest [ci, p, mb, mi] = blk[s(fw), p, mb, j(fw)] with fw=ci*128+mi
        dest = blk[s, :, :, j]            # [256(fw), p, mb]
        dest = dest.transpose(1, 2, 0)    # [p, mb, fw]
        dest = dest.reshape(128, M // 128, 2, 128)  # [p, mb, ci, mi]
        out[pc * 256:(pc + 1) * 256] = (
            dest.transpose(2, 0, 1, 3).reshape(256, M)
        )
    return out


def _nat_tiled(x: np.ndarray) -> np.ndarray:
    """[m_loc, d] natural array -> [P, mc_n*d] tile layout [p, m*d+k]."""
    m_loc, d = x.shape
    return np.ascontiguousarray(
        x.reshape(m_loc // P, P, d).transpose(1, 0, 2).reshape(P, -1)
    )


def _mT_tiled(xT: np.ndarray) -> np.ndarray:
    """[K, M] transposed array -> [P, M//P * K] m-major tile layout
    [p, mb*(K) + c*P + mi] = xT[c*P+p, mb*P+mi]."""
    K, M = xT.shape
    return np.ascontiguousarray(
        xT.reshape(K // P, P, M // P, P).transpose(1, 2, 0, 3).reshape(P, -1)
    )


def _kT_tiled(xT: np.ndarray) -> np.ndarray:
    """[K, cols] transposed array -> [P, (K//P)*cols] layout [p, c*cols+n]."""
    K, cols = xT.shape
    return np.ascontiguousarray(
        xT.reshape(K // P, P, cols).transpose(1, 0, 2).reshape(P, -1)
    )


def make_in_maps(image_embeddings: np.ndarray, text_embeddings: np.ndarray):
    n_global, d = image_embeddings.shape
    m_loc = n_global // W
    fp8 = ml_dtypes.float8_e4m3

    tn = text_embeddings / np.maximum(
        np.linalg.norm(text_embeddings, axis=1, keepdims=True), 1e-12
    )
    tn16 = (tn * TXT_PRESCALE).astype(np.float32)
    txtT_q = np.ascontiguousarray(tn16.T).astype(fp8)
    # txtT block-tiled: rows g*128+p, cols c*gw+n  (gw = m_loc)
    txtT_t = np.concatenate(
        [_kT_tiled(txtT_q[:, g * m_loc:(g + 1) * m_loc]) for g in range(W)],
        axis=0,
    )

    maps = []
    for k in range(W):
        sl = slice(k * m_loc, (k + 1) * m_loc)
        aT_q = np.ascontiguousarray(
            image_embeddings[sl].T.astype(np.float32)
        ).astype(fp8)
        maps.append({
            "img": _nat_tiled(image_embeddings[sl].astype(fp8)),
            "txtn": _nat_tiled(tn16[sl].astype(fp8)),
            "imgT": _mT_tiled(_interleave_weights(aT_q)),
            "txtT": txtT_t,
        })
    return maps


def kernel(image_embeddings: np.ndarray, text_embeddings: np.ndarray) -> np.ndarray:
    from concourse.bass_utils import run_bass_kernel_spmd

    n_global, d = image_embeddings.shape
    key = (n_global, d)
    if key not in _CACHE:
        _CACHE[key] = build_bass(n_global, d)
    nc = _CACHE[key]

    in_maps = make_in_maps(
        np.asarray(image_embeddings, np.float32),
        np.asarray(text_embeddings, np.float32),
    )
    res = run_bass_kernel_spmd(nc, in_maps, core_ids=list(range(W)))
    total = sum(float(r["partial"][0, 0]) for r in res.results)
    return np.asarray(total, dtype=np.float32)
